# revision 1
# baseline (speedup 1.0000x reference)
"""Trainium2 Bass kernel for nn_CustomLSTM_8461085573201 (raw-bass).

out = tanh(inputs[:, -1, :] @ kernel + bias); device work per core is a
[256,64] x [256,128] matmul tile + bias + tanh (2D shard: batch x4, units x2).

v3 over v2: the input is split across BOTH hardware DMA queues so the two
ring latencies overlap and mm1 can start while mm2's operands are still
in flight:
  SP  queue: da = [x0 | w0 | bias]  [128, 193]
  ACT queue: db = [x1 | w1]         [128, 192]
PE waits da -> mm1, waits db -> mm2 (PSUM accumulate).  ACT: tanh with
per-partition bias column, out-DMA dispatch in program order, final wait,
then sem restore (required: the runtime does not reset semaphores between
NEFF executions).
"""

import sys

sys.path.insert(0, "/opt/trn_rl_repo")

import numpy as np

import concourse.bass as bass
from concourse import mybir
from concourse.bass_utils import run_bass_kernel_spmd

B, T, F, U = 256, 512, 256, 256
N_CORES = 8
RB, CU = 4, 2              # batch split x unit split
BS, US = B // RB, U // CU  # 64, 128
PA = BS + US + 1           # 193 (x0|w0|bias)
PB = BS + US               # 192 (x1|w1)
FP32 = mybir.dt.float32
BF16 = mybir.dt.bfloat16

_cached_nc = None


def _build_nc() -> bass.Bass:
    orig_barrier = bass.Bass.all_engine_barrier
    bass.Bass.all_engine_barrier = lambda self, **kw: None
    try:
        nc = bass.Bass()
    finally:
        bass.Bass.all_engine_barrier = orig_barrier

    da = nc.declare_dram_parameter("da", [128, PA], FP32, isOutput=False)
    db = nc.declare_dram_parameter("db", [128, PB], FP32, isOutput=False)
    outT = nc.declare_dram_parameter("outT", [US, BS], BF16, isOutput=True)

    ta = nc.alloc_sbuf_tensor("ta", [128, PA], FP32)
    tb = nc.alloc_sbuf_tensor("tb", [128, PB], FP32)
    ot = nc.alloc_sbuf_tensor("ot", [US, BS], BF16)
    p = nc.alloc_psum_tensor("p", [US, BS], FP32)

    sa1 = nc.alloc_semaphore("dma_a1")
    sa2 = nc.alloc_semaphore("dma_a2")
    sb1 = nc.alloc_semaphore("dma_b1")
    sb2 = nc.alloc_semaphore("dma_b2")
    pe_sem = nc.alloc_semaphore("pe_done")
    out_sem = nc.alloc_semaphore("dma_out")

    aap = ta.ap()
    bap = tb.ap()
    x0 = aap[:, 0:BS]
    w0 = aap[:, BS : BS + US]
    bias_col = aap[:, BS + US : BS + US + 1]
    x1 = bap[:, 0:BS]
    w1 = bap[:, BS : BS + US]

    act_sem = nc.alloc_semaphore("act_done")

    # Each input tensor is row-split across BOTH rings, ta's halves
    # queued first in each ring: ta's 128 packets (8/engine) run
    # uncontended so mm1 starts earlier; tb's land under mm1's runtime.
    nc.sync.dma_start(out=aap[0:64, :], in_=da.ap()[0:64, :]).then_inc(sa1, 16)
    nc.sync.dma_start(out=bap[0:64, :], in_=db.ap()[0:64, :]).then_inc(sb1, 16)
    nc.scalar.dma_start(out=aap[64:128, :], in_=da.ap()[64:128, :]).then_inc(
        sa2, 16
    )
    nc.scalar.dma_start(out=bap[64:128, :], in_=db.ap()[64:128, :]).then_inc(
        sb2, 16
    )
    nc.tensor.wait_ge(sa1, 16)

    # Dummy activation at ACT stream start: forces walrus to emit the
    # 1.3us ACT_TABLE_LOAD here, overlapping the input DMA, instead of
    # after the pe_done wait (where it sits on the critical path).
    nc.scalar.activation(
        ot.ap()[0:1, 0:1], ot.ap()[0:1, 0:1], mybir.ActivationFunctionType.Tanh
    )

    # Waits are EMBEDDED in the consuming instruction (one each; walrus
    # allows a single fused wait).  Standalone EventSemaphore waits are
    # not enough: relaxed ordering mode lets the DMA dispatch hoist past
    # prior compute on the same engine (observed: out-DMA shipped stale
    # SBUF before ACTIVATE wrote it).
    mm1 = nc.tensor.matmul(p.ap(), w0, x0, start=True, stop=False)
    mm1._wait_ge(sa2, 16)
    nc.tensor.wait_ge(sb1, 16)
    mm2 = nc.tensor.matmul(p.ap(), w1, x1, start=False, stop=True)
    mm2._wait_ge(sb2, 16)
    mm2.then_inc(pe_sem, 1)

    act = nc.scalar.activation(
        ot.ap(), p.ap(), mybir.ActivationFunctionType.Tanh, bias=bias_col
    )
    act._wait_ge(pe_sem, 1)
    act.then_inc(act_sem, 1)
    # Split the out-DMA by PARTITION ROWS, not columns: each SBUF
    # partition row is one DMA packet regardless of size, so a column
    # split doubles the packet count (both queues touch all 128 rows)
    # while a row split halves per-queue packets with both rings parallel.
    HP = US // 2
    odma0 = nc.scalar.dma_start(out=outT.ap()[0:HP, :], in_=ot.ap()[0:HP, :])
    odma0._wait_ge(act_sem, 1)
    odma0.then_inc(out_sem, 16)
    odma1 = nc.sync.dma_start(out=outT.ap()[HP:US, :], in_=ot.ap()[HP:US, :])
    odma1._wait_ge(act_sem, 1)
    odma1.then_inc(out_sem, 16)
    nc.scalar.wait_ge(out_sem, 32)
    # Restore sems to 0 for the next NEFF execution (the runtime does not
    # reset them; stale values would let every wait fall through).  One
    # range clear: the five sems are allocated contiguously.
    nums = sorted(
        h.num for h in (sa1, sa2, sb1, sb2, pe_sem, act_sem, out_sem)
    )
    assert nums == list(range(nums[0], nums[0] + 7))
    nc.scalar.sem_clear(range(nums[0], nums[-1] + 1))
    return nc


def _get_nc() -> bass.Bass:
    global _cached_nc
    if _cached_nc is None:
        _cached_nc = _build_nc()
    return _cached_nc


def _pack_inputs(inputs, kernel, bias):
    x_last = np.ascontiguousarray(inputs[:, -1, :], dtype=np.float32)  # [B, F]
    xT = np.ascontiguousarray(x_last.T)                                # [F, B]
    w = np.asarray(kernel, dtype=np.float32)
    b = np.asarray(bias, dtype=np.float32)

    in_maps = []
    for core in range(N_CORES):
        bi, ui = divmod(core, CU)
        bs = slice(bi * BS, (bi + 1) * BS)
        us = slice(ui * US, (ui + 1) * US)
        da = np.empty((128, PA), dtype=np.float32)
        da[:, 0:BS] = xT[0:128, bs]
        da[:, BS : BS + US] = w[0:128, us]
        da[:, BS + US] = b[us]
        db = np.empty((128, PB), dtype=np.float32)
        db[:, 0:BS] = xT[128:256, bs]
        db[:, BS : BS + US] = w[128:256, us]
        in_maps.append({"da": da, "db": db})
    return in_maps


def kernel(inputs: np.ndarray, kernel: np.ndarray, bias: np.ndarray) -> np.ndarray:
    in_maps = _pack_inputs(inputs, kernel, bias)
    res = run_bass_kernel_spmd(_get_nc(), in_maps, list(range(N_CORES)))

    out = np.empty((B, U), dtype=np.float32)
    for core in range(N_CORES):
        bi, ui = divmod(core, CU)
        out[bi * BS : (bi + 1) * BS, ui * US : (ui + 1) * US] = np.asarray(
            res.results[core]["outT"], dtype=np.float32
        ).T
    return out

